# revision 23
# baseline (speedup 1.0000x reference)
"""GCLConv (GNN message passing) Trainium2 kernel — 8-core edge-parallel, v2.

Strategy (v2):
  - Host: sort edges by (col-half, dest-window); shard by destination node
    range across 8 cores (6272 nodes/core) => no cross-core reduction.
  - Row-side gather ELIMINATED: rows of a tile live in one 128-node window,
    so the layer-1 row term eW1t.T @ h[row] == A_w.T @ ST with
    A_w = h_win @ eW1top (one matmul per window, resident bf16) and ST a
    host-precomputed one-hot [node-in-win x edge] streamed from HBM.
  - 512-wide chunks (4 edge tiles) through the edge MLP: 2 silus per
    chunk on ACT ([128,512] PSUM->SBUF), batched attention tanh [128,k].
  - One-hot S built on DVE with fused tensor_scalar (is_equal [x mult]):
    per chunk 2 sub-tiles use ef=m2E*att on DVE + plain S; 2 sub-tiles use
    a Pool PSUM->SBUF copy of m2E + S_att (att folded into S) to balance
    DVE/Pool load.
  - Aggregation matmul emits aggT [H, win] directly (lhsT=ef, rhs=S), so
    the node phase needs no PE transposes; node MLP runs 4 windows wide.
  - sigmoid via tanh (one ACT table set with Silu/Copy): att' = tanh+1 and
    the 0.5 is folded into nW1bot (= nW1[D:] / NORM / 2).
"""
import sys

sys.path.insert(0, "/opt/trn_rl_repo")

import numpy as np
import ml_dtypes

import concourse.bass as bass
import concourse.bacc as bacc
import concourse.mybir as mybir
import concourse.tile as tile
from concourse import bass_utils

BF16 = ml_dtypes.bfloat16

N = 50000
E = 800000
D = 128
H = 128
P = 128
NCORES = 8
WIN = 128                  # nodes per aggregation window
NW = 49                    # windows per core
SHARD = WIN * NW           # 6272 nodes per core
NPAD = SHARD * NCORES      # 50176
COL_SPLIT = 25088          # col gather table split (both halves < 32768)
COL_HI = NPAD - COL_SPLIT  # 25088
GB = 32                    # max tiles per gather/stream batch
CHUNK = 4                  # tiles per compute chunk (512 edges)
NORM = 100.0

FP32 = mybir.dt.float32
BF = mybir.dt.bfloat16
I16 = mybir.dt.int16


def _idx_layout(idx_flat: np.ndarray) -> np.ndarray:
    """Pack int16 indices into the SWDGE layout [128, n/16]:
    index i -> partition i%16, col i//16, replicated across 8 groups."""
    n = idx_flat.shape[0]
    assert n % 16 == 0
    arr = idx_flat.reshape(n // 16, 16).T.astype(np.int16)  # [16, n/16]
    return np.tile(arr, (8, 1))                             # [128, n/16]


def _preprocess(h: np.ndarray, edge_index: np.ndarray):
    """Build per-core edge tiles + metadata. Returns host data dict."""
    row = np.asarray(edge_index[0], dtype=np.int64)
    col = np.asarray(edge_index[1], dtype=np.int64)

    core_of = row // SHARD
    half_of = (col >= COL_SPLIT).astype(np.int64)
    win_of = (row % SHARD) // WIN

    counts = np.zeros((NCORES, 2, NW), dtype=np.int64)
    np.add.at(counts, (core_of, half_of, win_of), 1)
    tiles_per_group = np.maximum(1, -(-counts // P))        # ceil, min 1
    T_hw = tiles_per_group.max(axis=0)                      # [2, NW] uniform
    NT = int(T_hw.sum())

    col_idx = np.empty((NCORES, 128, NT * 8), dtype=np.int16)
    rel_row = np.empty((NCORES, 128, NT), dtype=np.float32)
    ST = np.empty((NCORES, 128, NT * P), dtype=BF16)
    iota = np.arange(P, dtype=np.float32)[:, None]
    for k in range(NCORES):
        m = core_of == k
        rk, ck, hk, wk = row[m] - k * SHARD, col[m], half_of[m], win_of[m]
        order = np.lexsort((wk, hk))
        rk, ck, hk, wk = rk[order], ck[order], hk[order], wk[order]
        cnt = np.zeros((2, NW), dtype=np.int64)
        np.add.at(cnt, (hk, wk), 1)
        cols_l, rel_l = [], []
        pos = 0
        for hf in range(2):
            for w in range(NW):
                c = int(cnt[hf, w])
                npad_e = int(T_hw[hf, w]) * P - c
                c_g = ck[pos:pos + c] - hf * COL_SPLIT
                rel_g = (rk[pos:pos + c] % WIN).astype(np.float32)
                pos += c
                if npad_e:
                    c_g = np.concatenate([c_g, np.zeros(npad_e, np.int64)])
                    rel_g = np.concatenate(
                        [rel_g, np.full(npad_e, 255.0, np.float32)])
                cols_l.append(c_g)
                rel_l.append(rel_g)
        c_all = np.concatenate(cols_l)
        rel_all = np.concatenate(rel_l)
        assert c_all.shape[0] == NT * P
        col_idx[k] = _idx_layout(c_all.astype(np.int16))
        rel_row[k] = rel_all.reshape(NT, P).T.astype(np.float32)
        ST[k] = (iota == rel_all[None, :]).astype(BF16)

    # gather tables (col side only)
    h_pad = np.zeros((NPAD, D), dtype=np.float32)
    h_pad[:N] = h
    h_bf = h_pad.astype(BF16)
    hA = np.ascontiguousarray(h_bf[:COL_SPLIT])
    hB = np.ascontiguousarray(h_bf[COL_SPLIT:])

    # node-phase buffers per core
    hsh = h_pad.reshape(NCORES, NW, WIN, D)
    h_own = np.ascontiguousarray(
        hsh.transpose(0, 2, 1, 3).reshape(NCORES, WIN, NW * D))  # [128, w*128+d]
    hT = np.ascontiguousarray(
        hsh.transpose(0, 3, 1, 2).reshape(NCORES, D, NW * WIN)).astype(BF16)

    return dict(NT=NT, T_hw=T_hw, col_idx=col_idx, rel_row=rel_row, ST=ST,
                hA=hA, hB=hB, h_own=h_own, hT=hT)


def _build(nc: bass.Bass, NT: int, T_hw: np.ndarray,
           act_silu, act_tanh):
    """Emit the SPMD program. T_hw: [2, NW] tiles per (half, window)."""
    dt = nc.dram_tensor
    hA_t = dt("hA", [COL_SPLIT, D], BF, kind="ExternalInput")
    hB_t = dt("hB", [COL_HI, D], BF, kind="ExternalInput")
    cidx_t = dt("col_idx", [128, NT * 8], I16, kind="ExternalInput")
    rel_t = dt("rel_row", [128, NT], FP32, kind="ExternalInput")
    ST_t = dt("ST", [128, NT * P], BF, kind="ExternalInput")
    hown_t = dt("h_own", [WIN, NW * D], FP32, kind="ExternalInput")
    hT_t = dt("hT", [D, NW * WIN], BF, kind="ExternalInput")
    # weights / consts (replicated)
    eW1t_t = dt("eW1top", [D, H], BF, kind="ExternalInput")
    eW1b_t = dt("eW1bot", [D, H], BF, kind="ExternalInput")
    eW2_t = dt("eW2", [H, H], BF, kind="ExternalInput")
    aW_t = dt("aW_col", [H, 1], BF, kind="ExternalInput")
    nW1t_t = dt("nW1top", [D, H], BF, kind="ExternalInput")
    nW1b_t = dt("nW1bot", [H, H], BF, kind="ExternalInput")      # / (2*NORM)
    nW2_t = dt("nW2", [H, D], BF, kind="ExternalInput")
    eb1_t = dt("eb1", [H, 1], FP32, kind="ExternalInput")
    eb2_t = dt("eb2", [H, 1], FP32, kind="ExternalInput")
    nb1_t = dt("nb1", [H, 1], FP32, kind="ExternalInput")
    jconst_t = dt("jconst", [P, WIN], BF, kind="ExternalInput")
    identb_t = dt("ident_bf", [P, P], BF, kind="ExternalInput")
    ab_t = dt("ab_c", [P, 1], FP32, kind="ExternalInput")  # 0.5*ab

    out_t = dt("out", [SHARD, D], FP32, kind="ExternalOutput")

    # ---- schedule: per-tile metadata, then chunks of <=4 consecutive
    # tiles spanning window groups (within one gather half).
    tile_meta = []  # (hf, w, gfirst, glast) per global tile index
    for hf in range(2):
        for w in range(NW):
            nt = int(T_hw[hf, w])
            for i in range(nt):
                tile_meta.append((hf, w, i == 0, i == nt - 1))
    assert len(tile_meta) == NT
    TA = int(T_hw[0].sum())

    chunks = []   # (hf, t0, k)
    for lo, hi in ((0, TA), (TA, NT)):
        t = lo
        while t < hi:
            k = min(CHUNK, hi - t)
            chunks.append((0 if lo == 0 else 1, t, k))
            t += k

    # pack chunks into gather/stream batches (same half, <= GB tiles)
    batches = []      # (t0, ntiles, half)
    chunk_batch = {}  # chunk idx -> (batch idx, tile offset within batch)
    first_chunk_of_batch = {}
    for ci, (hf, t0, k) in enumerate(chunks):
        if (not batches or batches[-1][2] != hf
                or batches[-1][1] + k > GB):
            batches.append((t0, 0, hf))
            first_chunk_of_batch[len(batches) - 1] = ci
        b0, nb, _ = batches[-1]
        chunk_batch[ci] = (len(batches) - 1, nb)
        batches[-1] = (b0, nb + k, hf)

    with tile.TileContext(nc) as tc:
        with (
            tc.tile_pool(name="const", bufs=1) as cp,
            tc.tile_pool(name="gather", bufs=3) as gp,
            tc.tile_pool(name="work", bufs=4) as wp,
            tc.tile_pool(name="psA", bufs=2, space="PSUM") as psa,
            tc.tile_pool(name="psB", bufs=2, space="PSUM") as psb,
            tc.tile_pool(name="psC", bufs=1, space="PSUM") as psc,
            tc.tile_pool(name="psG", bufs=1, space="PSUM") as psg,
                    ):
            # --- resident uploads ---
            def up(shape, dtype, src, tag):
                t = cp.tile(shape, dtype, tag=tag)
                nc.sync.dma_start(out=t[:], in_=src[:])
                return t

            # upload order matters: gather prerequisites first so the first
            # gather/stream batches start ASAP; h_own (node phase only) last.
            cidx = up([128, NT * 8], I16, cidx_t, "cidx")

            # --- gather/stream batches ---
            gtiles = {}

            def emit_batch(bi):
                b0, nb, half = batches[bi]
                src = hA_t if half == 0 else hB_t
                ct = gp.tile([128, GB * P], BF, tag="gcol")
                nc.gpsimd.dma_gather(
                    out_ap=ct[:, :nb * P].rearrange("p (a n) -> p a n", a=1),
                    in_ap=src[:],
                    idxs_ap=cidx[:, b0 * 8:(b0 + nb) * 8],
                    num_idxs=nb * P,
                    num_idxs_reg=nb * P,
                    elem_size=D,
                    transpose=True,
                    single_packet=False,
                )
                st = gp.tile([128, GB * P], BF, tag="gst")
                nc.sync.dma_start(out=st[:, :nb * P],
                                  in_=ST_t[:, b0 * P:(b0 + nb) * P])
                gtiles[bi] = (ct, st)

            emit_batch(0)
            emit_batch(1)

            hT = up([D, NW * WIN], BF, hT_t, "hT")
            eW1t = up([D, H], BF, eW1t_t, "eW1t")
            eW1b = up([D, H], BF, eW1b_t, "eW1b")
            eW2 = up([H, H], BF, eW2_t, "eW2")
            aW = up([H, 1], BF, aW_t, "aW")
            eb1 = up([H, 1], FP32, eb1_t, "eb1")
            eb2 = up([H, 1], FP32, eb2_t, "eb2")
            rel = up([128, NT], FP32, rel_t, "rel")
            jconst = up([P, WIN], BF, jconst_t, "jconst")
            ident_bf = up([P, P], BF, identb_t, "identb")
            ab_c = up([P, 1], FP32, ab_t, "ab")
            nW1t = up([D, H], BF, nW1t_t, "nW1t")
            nW1b = up([H, H], BF, nW1b_t, "nW1b")
            nW2 = up([H, D], BF, nW2_t, "nW2")
            nb1 = up([H, 1], FP32, nb1_t, "nb1")

            aggT32 = cp.tile([H, NW * WIN], FP32, tag="aggT32")
            aggTbf = cp.tile([H, NW * WIN], BF, tag="aggTbf")
            h_own = cp.tile([WIN, NW * D], FP32, tag="hown")

            mult = mybir.AluOpType.mult
            iseq = mybir.AluOpType.is_equal

            # --- A_w = h_win @ eW1top (bf16 resident), 4 windows per
            # PSUM tile + one wide copy to keep the prologue short ---
            Aw = {}
            for q0 in range(0, NW, 4):
                qn = min(4, NW - q0)
                aps = psa.tile([128, 512], FP32, space="PSUM", tag="ps1")
                for qi in range(qn):
                    wsl = slice((q0 + qi) * WIN, (q0 + qi + 1) * WIN)
                    nc.tensor.matmul(aps[:, qi * H:(qi + 1) * H],
                                     lhsT=hT[:, wsl], rhs=eW1t[:],
                                     start=True, stop=True)
                a_sb = cp.tile([128, 512], BF, tag=f"Aw{q0}")
                nc.vector.tensor_copy(a_sb[:, :qn * H], aps[:, :qn * H])
                for qi in range(qn):
                    Aw[q0 + qi] = a_sb[:, qi * H:(qi + 1) * H]

            def get_Aw(w):
                return Aw[w]
            # --- software pipeline: stage1(i) | stage2a(i-1) | stage2b(i-2)
            state = {}      # chunk idx -> dict of live tiles
            pagg_of_group = {}  # (hf, w) -> paggT tile

            def stage1(ci):
                hf, t0, k = chunks[ci]
                bi, off = chunk_batch[ci]
                if (first_chunk_of_batch.get(bi) == ci
                        and bi + 1 < len(batches) and bi + 1 not in gtiles):
                    emit_batch(bi + 1)
                ct, st = gtiles[bi]
                cs = slice(off * P, (off + k) * P)
                W = k * P
                ps1 = psa.tile([128, 512], FP32, space="PSUM", tag="ps1")
                # one A_w matmul per window-run within the chunk
                j = 0
                while j < k:
                    w = tile_meta[t0 + j][1]
                    j2 = j
                    while j2 < k and tile_meta[t0 + j2][1] == w:
                        j2 += 1
                    nc.tensor.matmul(
                        ps1[:, j * P:j2 * P], lhsT=get_Aw(w),
                        rhs=st[:, off * P + j * P:off * P + j2 * P],
                        start=True, stop=False)
                    j = j2
                nc.tensor.matmul(ps1[:, :W], lhsT=eW1b[:], rhs=ct[:, cs],
                                 start=False, stop=True)
                m1 = wp.tile([128, 512], BF, tag="m1")
                nc.scalar.activation(m1[:, :W], ps1[:, :W], act_silu,
                                     bias=eb1[:])
                ps2 = psa.tile([128, 512], FP32, space="PSUM", tag="ps2")
                nc.tensor.matmul(ps2[:, :W], lhsT=eW2[:], rhs=m1[:, :W],
                                 start=True, stop=True)
                m2 = wp.tile([128, 512], BF, tag="m2")
                nc.scalar.activation(m2[:, :W], ps2[:, :W], act_silu,
                                     bias=eb2[:])
                state[ci] = dict(m2=m2)

            def stage2a(ci):
                hf, t0, k = chunks[ci]
                stt = state[ci]
                m2 = stt["m2"]
                ps3 = psb.tile([128, 512], FP32, space="PSUM", tag="ps3")
                attp = psc.tile([128, 8], FP32, space="PSUM", tag="attp")
                for j in range(k):
                    jsl = slice(j * P, (j + 1) * P)
                    nc.tensor.matmul(ps3[:, jsl], lhsT=m2[:, jsl],
                                     rhs=ident_bf[:], start=True, stop=True)
                    nc.tensor.matmul(attp[:, j:j + 1], lhsT=m2[:, jsl],
                                     rhs=aW[:], start=True, stop=True)
                att_t = wp.tile([128, 8], FP32, tag="att_t")
                nc.scalar.activation(att_t[:, :k], attp[:, :k], act_tanh,
                                     bias=ab_c[:], scale=0.5)
                att4 = wp.tile([128, 8], FP32, tag="att4")
                nc.vector.tensor_scalar_add(att4[:, :k], att_t[:, :k], 1.0)
                # one wide edge-feature scale: ef4 = m2E * att (per-block
                # broadcast of att4 along the free dim)
                W = k * P
                ef4 = wp.tile([128, 512], BF, tag="ef4")
                nc.vector.tensor_tensor(
                    out=ef4[:, :W].rearrange("p (j c) -> p j c", j=k),
                    in0=ps3[:, :W].rearrange("p (j c) -> p j c", j=k),
                    in1=att4[:, :k].to_broadcast([128, k, P]),
                    op=mult)
                Ss = []
                for j in range(k):
                    t_idx = t0 + j
                    if j % 2 == 0:
                        S = wp.tile([P, WIN], BF, tag="S")
                        nc.vector.tensor_scalar(
                            out=S[:], in0=jconst[:],
                            scalar1=rel[:, t_idx:t_idx + 1], scalar2=None,
                            op0=iseq)
                    else:
                        S = wp.tile([P, WIN], BF, tag="Sp")
                        nc.gpsimd.tensor_scalar(
                            out=S[:], in0=jconst[:],
                            scalar1=rel[:, t_idx:t_idx + 1], scalar2=None,
                            op0=iseq)
                    Ss.append(S)
                stt["ef4"] = ef4
                stt["Ss"] = Ss

            def stage2b(ci):
                hf, t0, k = chunks[ci]
                stt = state.pop(ci)
                ef4 = stt["ef4"]
                for j, S in enumerate(stt["Ss"]):
                    _, w, gfirst, glast = tile_meta[t0 + j]
                    if gfirst:
                        paggT = psg.tile([H, WIN], FP32, space="PSUM",
                                         tag="paggT")
                        pagg_of_group[(hf, w)] = paggT
                    paggT = pagg_of_group[(hf, w)]
                    nc.tensor.matmul(paggT[:], lhsT=ef4[:, j * P:(j + 1) * P],
                                     rhs=S[:], start=gfirst, stop=glast)
                    if glast:
                        del pagg_of_group[(hf, w)]
                        wsl = slice(w * WIN, (w + 1) * WIN)
                        if hf == 0:
                            nc.vector.tensor_copy(aggT32[:, wsl], paggT[:])
                        else:
                            nc.vector.tensor_add(
                                out=aggTbf[:, wsl], in0=aggT32[:, wsl],
                                in1=paggT[:])

            # --- node phase pass (emitted interleaved, 4 windows each) ---
            def node_pass(w0, wg):
                W2 = wg * WIN
                fsl = slice(w0 * WIN, w0 * WIN + W2)
                psn1 = psc.tile([128, 512], FP32, space="PSUM", tag="attp")
                nc.tensor.matmul(psn1[:, :W2], lhsT=nW1t[:], rhs=hT[:, fsl],
                                 start=True, stop=False)
                nc.tensor.matmul(psn1[:, :W2], lhsT=nW1b[:],
                                 rhs=aggTbf[:, fsl], start=False, stop=True)
                y1 = wp.tile([128, 512], BF, tag="m1")
                nc.scalar.activation(y1[:, :W2], psn1[:, :W2], act_silu,
                                     bias=nb1[:])
                psn2 = psc.tile([128, 512], FP32, space="PSUM", tag="attp")
                for j in range(wg):
                    nc.tensor.matmul(psn2[:, j * D:(j + 1) * D],
                                     lhsT=y1[:, j * WIN:(j + 1) * WIN],
                                     rhs=nW2[:], start=True, stop=True)
                o_sb = wp.tile([128, 512], FP32, tag="osb")
                nc.vector.tensor_add(out=o_sb[:, :W2], in0=psn2[:, :W2],
                                     in1=h_own[:, w0 * D:w0 * D + W2])
                for j in range(wg):
                    nc.sync.dma_start(
                        out=out_t[(w0 + j) * WIN:(w0 + j + 1) * WIN, :],
                        in_=o_sb[:, j * D:(j + 1) * D])

            # node group g ready after the chunk finishing window
            # (1, w0+wg-1)'s group; map chunk idx -> node groups to emit
            def chunk_of_tile(t):
                for ci, (hf, t0, k) in enumerate(chunks):
                    if t0 <= t < t0 + k:
                        return ci
                raise AssertionError(t)

            cumB = TA
            last_tile_B = {}
            for w in range(NW):
                cumB += int(T_hw[1, w])
                last_tile_B[w] = cumB - 1
            node_after = {}
            w0 = 0
            while w0 < NW:
                wg = min(4, NW - w0)
                ci_g = chunk_of_tile(last_tile_B[w0 + wg - 1])
                node_after.setdefault(ci_g, []).append((w0, wg))
                w0 += wg

            NC = len(chunks)
            for i in range(NC + 4):
                if 0 <= i - 4:
                    for (nw0, nwg) in node_after.get(i - 4, []):
                        node_pass(nw0, nwg)
                if 0 <= i - 1 < NC:
                    stage2a(i - 1)
                if 0 <= i - 2 < NC:
                    stage2b(i - 2)
                if i < NC:
                    stage1(i)
                if i == 20:
                    nc.sync.dma_start(out=h_own[:], in_=hown_t[:])
    return nc


def _make_in_maps(prep, inputs):
    eW1 = np.asarray(inputs["eW1"], np.float32)
    aW = np.asarray(inputs["aW"], np.float32)
    nW1 = np.asarray(inputs["nW1"], np.float32)
    nb2 = np.asarray(inputs["nb2"], np.float32).reshape(D)
    jconst = np.broadcast_to(np.arange(WIN, dtype=np.float32)[None, :],
                             (P, WIN))
    common = {
        "hA": prep["hA"], "hB": prep["hB"],
        "eW1top": eW1[:D].astype(BF16), "eW1bot": eW1[D:].astype(BF16),
        "eW2": np.asarray(inputs["eW2"], np.float32).astype(BF16),
        "aW_col": aW.reshape(H, 1).astype(BF16),
        "nW1top": nW1[:D].astype(BF16),
        # att' = tanh+1 = 2*sigmoid  =>  fold the 0.5 in here with 1/NORM
        "nW1bot": (nW1[D:] / (2.0 * NORM)).astype(BF16),
        "nW2": np.asarray(inputs["nW2"], np.float32).astype(BF16),
        "eb1": np.asarray(inputs["eb1"], np.float32).reshape(H, 1),
        "eb2": np.asarray(inputs["eb2"], np.float32).reshape(H, 1),
        "nb1": np.asarray(inputs["nb1"], np.float32).reshape(H, 1),
        "jconst": np.ascontiguousarray(jconst).astype(BF16),
        "ident_bf": np.eye(P, dtype=np.float32).astype(BF16),
        # tanh form: sigmoid(x+ab) = 0.5*tanh(0.5x + 0.5ab) + 0.5
        "ab_c": np.full((P, 1), 0.5 * float(np.asarray(inputs["ab"]).ravel()[0]),
                        dtype=np.float32),
    }
    in_maps = []
    for k in range(NCORES):
        m = dict(common)
        m["col_idx"] = np.ascontiguousarray(prep["col_idx"][k])
        m["rel_row"] = np.ascontiguousarray(prep["rel_row"][k])
        m["ST"] = np.ascontiguousarray(prep["ST"][k])
        # residual carries the nb2 bias (saves the bias matmul on device)
        m["h_own"] = np.ascontiguousarray(
            prep["h_own"][k] + np.tile(nb2, NW)[None, :])
        m["hT"] = np.ascontiguousarray(prep["hT"][k])
        in_maps.append(m)
    return in_maps


_RUN_KW = {}


def kernel(**inputs) -> np.ndarray:
    h = np.asarray(inputs["h"], np.float32)
    prep = _preprocess(h, np.asarray(inputs["edge_index"]))

    nc = bacc.Bacc("TRN2", target_bir_lowering=False, debug=False,
                   num_devices=NCORES)
    _build(nc, prep["NT"], prep["T_hw"],
           act_silu=mybir.ActivationFunctionType.Silu,
           act_tanh=mybir.ActivationFunctionType.Tanh)
    nc.compile()

    in_maps = _make_in_maps(prep, inputs)
    res = bass_utils.run_bass_kernel_spmd(
        nc, in_maps, core_ids=list(range(NCORES)), **_RUN_KW)
    out = np.empty((NPAD, D), dtype=np.float32)
    for k in range(NCORES):
        out[k * SHARD:(k + 1) * SHARD] = np.asarray(res.results[k]["out"])
    kernel._last_results = res
    return out[:N]


# revision 34
# speedup vs baseline: 201.8332x; 201.8332x over previous
"""GCLConv (GNN message passing) Trainium2 kernel — 8-core edge-parallel, v2.

Strategy:
  - Host: sort edges by (col-half, dest-window); shard by destination node
    range across 8 cores (6272 nodes/core) => no cross-core reduction.
  - Row-side gather ELIMINATED: rows of a tile live in one 128-node window,
    so the layer-1 row term eW1t.T @ h[row] == A_w.T @ ST with
    A_w = h_win @ eW1top (one matmul per window, resident bf16) and ST a
    host-precomputed one-hot [node-in-win x edge] streamed from HBM at
    full DMA efficiency (vs per-edge 256B gather descriptors).
  - 512-wide chunks (4 edge tiles, spanning window groups within a gather
    half) through the edge MLP: 2 silus per chunk on ACT ([128,512]
    PSUM->SBUF); attention tanh batched [128,8] per chunk PAIR.
  - m2 transposed per sub-tile via identity matmul; one wide DVE
    tensor_tensor applies att to the whole chunk (ef4 = m2E * att with a
    per-block stride-0 broadcast AP). One-hot S built by fused DVE/Pool
    tensor_scalar(is_equal) against a per-edge rel scalar (255 sentinel
    for padding edges -> zero column).
  - Aggregation matmul emits aggT [H, win] directly (lhsT=ef4, rhs=S), so
    the node phase needs no PE transposes; node MLP runs 4 windows wide
    and is emitted interleaved as windows complete.
  - Explicit 4-deep software pipeline (stage1a/2a1/2a2/2b/stage1b) with
    per-tag PSUM pools sized to break WAR serialization; small first
    gather batches so compute starts ~12us in.
  - sigmoid via tanh (one ACT table set with Silu/Copy): att' = tanh+1 and
    the 0.5 is folded into nW1bot (= nW1[D:] / NORM / 2).
"""
import sys

sys.path.insert(0, "/opt/trn_rl_repo")

import numpy as np
import ml_dtypes

import concourse.bass as bass
import concourse.bacc as bacc
import concourse.mybir as mybir
import concourse.tile as tile
from concourse import bass_utils

BF16 = ml_dtypes.bfloat16

N = 50000
E = 800000
D = 128
H = 128
P = 128
NCORES = 8
WIN = 128                  # nodes per aggregation window
NW = 49                    # windows per core
SHARD = WIN * NW           # 6272 nodes per core
NPAD = SHARD * NCORES      # 50176
COL_SPLIT = 25088          # col gather table split (both halves < 32768)
COL_HI = NPAD - COL_SPLIT  # 25088
GB = 32                    # max tiles per gather/stream batch
CHUNK = 4                  # tiles per compute chunk (512 edges)
NORM = 100.0

FP32 = mybir.dt.float32
BF = mybir.dt.bfloat16
I16 = mybir.dt.int16


def _idx_layout(idx_flat: np.ndarray) -> np.ndarray:
    """Pack int16 indices into the SWDGE layout [128, n/16]:
    index i -> partition i%16, col i//16, replicated across 8 groups."""
    n = idx_flat.shape[0]
    assert n % 16 == 0
    arr = idx_flat.reshape(n // 16, 16).T.astype(np.int16)  # [16, n/16]
    return np.tile(arr, (8, 1))                             # [128, n/16]


def _preprocess(h: np.ndarray, edge_index: np.ndarray):
    """Build per-core edge tiles + metadata. Returns host data dict."""
    row = np.asarray(edge_index[0], dtype=np.int64)
    col = np.asarray(edge_index[1], dtype=np.int64)

    core_of = row // SHARD
    half_of = (col >= COL_SPLIT).astype(np.int64)
    win_of = (row % SHARD) // WIN

    counts = np.zeros((NCORES, 2, NW), dtype=np.int64)
    np.add.at(counts, (core_of, half_of, win_of), 1)
    tiles_per_group = np.maximum(1, -(-counts // P))        # ceil, min 1
    T_hw = tiles_per_group.max(axis=0)                      # [2, NW] uniform
    NT = int(T_hw.sum())

    col_idx = np.empty((NCORES, 128, NT * 8), dtype=np.int16)
    rel_row = np.empty((NCORES, 128, NT), dtype=np.float32)
    ST = np.empty((NCORES, 128, NT * P), dtype=BF16)
    iota = np.arange(P, dtype=np.float32)[:, None]
    for k in range(NCORES):
        m = core_of == k
        rk, ck, hk, wk = row[m] - k * SHARD, col[m], half_of[m], win_of[m]
        order = np.lexsort((wk, hk))
        rk, ck, hk, wk = rk[order], ck[order], hk[order], wk[order]
        cnt = np.zeros((2, NW), dtype=np.int64)
        np.add.at(cnt, (hk, wk), 1)
        cols_l, rel_l = [], []
        pos = 0
        for hf in range(2):
            for w in range(NW):
                c = int(cnt[hf, w])
                npad_e = int(T_hw[hf, w]) * P - c
                c_g = ck[pos:pos + c] - hf * COL_SPLIT
                rel_g = (rk[pos:pos + c] % WIN).astype(np.float32)
                pos += c
                if npad_e:
                    c_g = np.concatenate([c_g, np.zeros(npad_e, np.int64)])
                    rel_g = np.concatenate(
                        [rel_g, np.full(npad_e, 255.0, np.float32)])
                cols_l.append(c_g)
                rel_l.append(rel_g)
        c_all = np.concatenate(cols_l)
        rel_all = np.concatenate(rel_l)
        assert c_all.shape[0] == NT * P
        col_idx[k] = _idx_layout(c_all.astype(np.int16))
        rel_row[k] = rel_all.reshape(NT, P).T.astype(np.float32)
        ST[k] = (iota == rel_all[None, :]).astype(BF16)

    # gather tables (col side only)
    h_pad = np.zeros((NPAD, D), dtype=np.float32)
    h_pad[:N] = h
    h_bf = h_pad.astype(BF16)
    hA = np.ascontiguousarray(h_bf[:COL_SPLIT])
    hB = np.ascontiguousarray(h_bf[COL_SPLIT:])

    # node-phase buffers per core
    hsh = h_pad.reshape(NCORES, NW, WIN, D)
    h_own = np.ascontiguousarray(
        hsh.transpose(0, 2, 1, 3).reshape(NCORES, WIN, NW * D))  # [128, w*128+d]
    hT = np.ascontiguousarray(
        hsh.transpose(0, 3, 1, 2).reshape(NCORES, D, NW * WIN)).astype(BF16)

    return dict(NT=NT, T_hw=T_hw, col_idx=col_idx, rel_row=rel_row, ST=ST,
                hA=hA, hB=hB, h_own=h_own, hT=hT)


def _build(nc: bass.Bass, NT: int, T_hw: np.ndarray,
           act_silu, act_tanh):
    """Emit the SPMD program. T_hw: [2, NW] tiles per (half, window)."""
    dt = nc.dram_tensor
    hA_t = dt("hA", [COL_SPLIT, D], BF, kind="ExternalInput")
    hB_t = dt("hB", [COL_HI, D], BF, kind="ExternalInput")
    cidx_t = dt("col_idx", [128, NT * 8], I16, kind="ExternalInput")
    rel_t = dt("rel_row", [128, NT], FP32, kind="ExternalInput")
    ST_t = dt("ST", [128, NT * P], BF, kind="ExternalInput")
    hown_t = dt("h_own", [WIN, NW * D], FP32, kind="ExternalInput")
    hT_t = dt("hT", [D, NW * WIN], BF, kind="ExternalInput")
    # weights / consts (replicated)
    eW1t_t = dt("eW1top", [D, H], BF, kind="ExternalInput")
    eW1b_t = dt("eW1bot", [D, H], BF, kind="ExternalInput")
    eW2_t = dt("eW2", [H, H], BF, kind="ExternalInput")
    aW_t = dt("aW_col", [H, 1], BF, kind="ExternalInput")
    nW1t_t = dt("nW1top", [D, H], BF, kind="ExternalInput")
    nW1b_t = dt("nW1bot", [H, H], BF, kind="ExternalInput")      # / (2*NORM)
    nW2_t = dt("nW2", [H, D], BF, kind="ExternalInput")
    eb1_t = dt("eb1", [H, 1], FP32, kind="ExternalInput")
    eb2_t = dt("eb2", [H, 1], FP32, kind="ExternalInput")
    nb1_t = dt("nb1", [H, 1], FP32, kind="ExternalInput")
    jconst_t = dt("jconst", [P, WIN], BF, kind="ExternalInput")
    identb_t = dt("ident_bf", [P, P], BF, kind="ExternalInput")
    ab_t = dt("ab_c", [P, 1], FP32, kind="ExternalInput")  # 0.5*ab

    out_t = dt("out", [SHARD, D], FP32, kind="ExternalOutput")

    # ---- schedule: per-tile metadata, then chunks of <=4 consecutive
    # tiles spanning window groups (within one gather half).
    tile_meta = []  # (hf, w, gfirst, glast) per global tile index
    for hf in range(2):
        for w in range(NW):
            nt = int(T_hw[hf, w])
            for i in range(nt):
                tile_meta.append((hf, w, i == 0, i == nt - 1))
    assert len(tile_meta) == NT
    TA = int(T_hw[0].sum())

    chunks = []   # (hf, t0, k)
    for lo, hi in ((0, TA), (TA, NT)):
        t = lo
        while t < hi:
            k = min(CHUNK, hi - t)
            chunks.append((0 if lo == 0 else 1, t, k))
            t += k

    # pack chunks into gather/stream batches (same half); the first few
    # batches are small so the pipeline starts early, then full GB tiles.
    def batch_cap(bi):
        return (8, 8, 8, 8, 16, 16, 24)[bi] if bi < 7 else GB

    batches = []      # (t0, ntiles, half)
    chunk_batch = {}  # chunk idx -> (batch idx, tile offset within batch)
    first_chunk_of_batch = {}
    for ci, (hf, t0, k) in enumerate(chunks):
        if (not batches or batches[-1][2] != hf
                or batches[-1][1] + k > batch_cap(len(batches) - 1)):
            batches.append((t0, 0, hf))
            first_chunk_of_batch[len(batches) - 1] = ci
        b0, nb, _ = batches[-1]
        chunk_batch[ci] = (len(batches) - 1, nb)
        batches[-1] = (b0, nb + k, hf)

    with tile.TileContext(nc) as tc:
        with (
            tc.tile_pool(name="const", bufs=1) as cp,
            tc.tile_pool(name="gather", bufs=4) as gp,
            tc.tile_pool(name="work", bufs=4) as wp,
            tc.tile_pool(name="psA", bufs=2, space="PSUM") as psa,
            tc.tile_pool(name="psB", bufs=2, space="PSUM") as psb,
            tc.tile_pool(name="psC", bufs=1, space="PSUM") as psc,
            tc.tile_pool(name="psG", bufs=1, space="PSUM") as psg,
                    ):
            # --- resident uploads ---
            def up(shape, dtype, src, tag):
                t = cp.tile(shape, dtype, tag=tag)
                nc.sync.dma_start(out=t[:], in_=src[:])
                return t

            # upload order matters: gather prerequisites first so the first
            # gather/stream batches start ASAP; h_own (node phase only) last.
            CSPLIT = 8 * GB * 8
            cidx = cp.tile([128, NT * 8], I16, tag="cidx")
            nc.sync.dma_start(out=cidx[:, :CSPLIT], in_=cidx_t[:, :CSPLIT])
            hT = up([D, NW * WIN], BF, hT_t, "hT")
            eW1t = up([D, H], BF, eW1t_t, "eW1t")
            eW1b = up([D, H], BF, eW1b_t, "eW1b")
            eW2 = up([H, H], BF, eW2_t, "eW2")
            eb1 = up([H, 1], FP32, eb1_t, "eb1")
            eb2 = up([H, 1], FP32, eb2_t, "eb2")

            # --- gather/stream batches ---
            gtiles = {}

            def emit_batch(bi):
                b0, nb, half = batches[bi]
                src = hA_t if half == 0 else hB_t
                ct = gp.tile([128, GB * P], BF, tag="gcol")
                nc.gpsimd.dma_gather(
                    out_ap=ct[:, :nb * P].rearrange("p (a n) -> p a n", a=1),
                    in_ap=src[:],
                    idxs_ap=cidx[:, b0 * 8:(b0 + nb) * 8],
                    num_idxs=nb * P,
                    num_idxs_reg=nb * P,
                    elem_size=D,
                    transpose=True,
                    single_packet=False,
                )
                st = gp.tile([128, GB * P], BF, tag="gst")
                nc.sync.dma_start(out=st[:, :nb * P],
                                  in_=ST_t[:, b0 * P:(b0 + nb) * P])
                gtiles[bi] = (ct, st)

            emit_batch(0)
            emit_batch(1)
            emit_batch(2)
            nc.sync.dma_start(out=cidx[:, CSPLIT:], in_=cidx_t[:, CSPLIT:])

            aW = up([H, 1], BF, aW_t, "aW")
            rel = up([128, NT], FP32, rel_t, "rel")
            jconst = up([P, WIN], BF, jconst_t, "jconst")
            ident_bf = up([P, P], BF, identb_t, "identb")
            ab_c = up([P, 1], FP32, ab_t, "ab")
            nW1t = up([D, H], BF, nW1t_t, "nW1t")
            nW1b = up([H, H], BF, nW1b_t, "nW1b")
            nW2 = up([H, D], BF, nW2_t, "nW2")
            nb1 = up([H, 1], FP32, nb1_t, "nb1")

            aggT32 = cp.tile([H, NW * WIN], FP32, tag="aggT32")
            aggTbf = cp.tile([H, NW * WIN], BF, tag="aggTbf")
            h_own = cp.tile([WIN, NW * D], FP32, tag="hown")

            mult = mybir.AluOpType.mult
            iseq = mybir.AluOpType.is_equal

            # --- A_w = h_win @ eW1top (bf16 resident), 4 windows per
            # PSUM tile + one wide copy to keep the prologue short ---
            Aw = {}
            for q0 in range(0, NW, 4):
                qn = min(4, NW - q0)
                aps = psa.tile([128, 512], FP32, space="PSUM", tag="ps1")
                for qi in range(qn):
                    wsl = slice((q0 + qi) * WIN, (q0 + qi + 1) * WIN)
                    nc.tensor.matmul(aps[:, qi * H:(qi + 1) * H],
                                     lhsT=hT[:, wsl], rhs=eW1t[:],
                                     start=True, stop=True)
                a_sb = cp.tile([128, 512], BF, tag=f"Aw{q0}")
                nc.vector.tensor_copy(a_sb[:, :qn * H], aps[:, :qn * H])
                for qi in range(qn):
                    Aw[q0 + qi] = a_sb[:, qi * H:(qi + 1) * H]

            def get_Aw(w):
                return Aw[w]
            # --- software pipeline: stage1(i) | stage2a(i-1) | stage2b(i-2)
            state = {}      # chunk idx -> dict of live tiles
            pagg_of_group = {}  # (hf, w) -> paggT tile

            def stage1a(ci):
                hf, t0, k = chunks[ci]
                bi, off = chunk_batch[ci]
                if first_chunk_of_batch.get(bi) == ci:
                    for nb_i in (bi + 1, bi + 2, bi + 3):
                        if nb_i < len(batches) and nb_i not in gtiles:
                            emit_batch(nb_i)
                ct, st = gtiles[bi]
                cs = slice(off * P, (off + k) * P)
                W = k * P
                ps1 = psa.tile([128, 512], FP32, space="PSUM", tag="ps1")
                # one A_w matmul per window-run within the chunk
                j = 0
                while j < k:
                    w = tile_meta[t0 + j][1]
                    j2 = j
                    while j2 < k and tile_meta[t0 + j2][1] == w:
                        j2 += 1
                    nc.tensor.matmul(
                        ps1[:, j * P:j2 * P], lhsT=get_Aw(w),
                        rhs=st[:, off * P + j * P:off * P + j2 * P],
                        start=True, stop=False)
                    j = j2
                nc.tensor.matmul(ps1[:, :W], lhsT=eW1b[:], rhs=ct[:, cs],
                                 start=False, stop=True)
                state[ci] = dict(ps1=ps1, ct=ct, cs=cs)

            def stage1b(ci):
                hf, t0, k = chunks[ci]
                W = k * P
                stt = state[ci]
                ps1 = stt.pop("ps1")
                m1 = wp.tile([128, 512], BF, tag="m1")
                nc.scalar.activation(m1[:, :W], ps1[:, :W], act_silu,
                                     bias=eb1[:])
                ps2 = psa.tile([128, 512], FP32, space="PSUM", tag="ps2")
                nc.tensor.matmul(ps2[:, :W], lhsT=eW2[:], rhs=m1[:, :W],
                                 start=True, stop=True)
                m2 = wp.tile([128, 512], BF, tag="m2")
                nc.scalar.activation(m2[:, :W], ps2[:, :W], act_silu,
                                     bias=eb2[:])
                stt["m2"] = m2

            pair_state = {}

            def stage2a1(ci):
                """ps3 + attp matmuls; attp is shared per chunk pair."""
                hf, t0, k = chunks[ci]
                stt = state[ci]
                m2 = stt["m2"]
                ps3 = psb.tile([128, 512], FP32, space="PSUM", tag="ps3")
                pi, base = ci // 2, (ci % 2) * CHUNK
                if ci % 2 == 0:
                    attp = psc.tile([128, 8], FP32, space="PSUM", tag="attp")
                    pair_state[pi] = attp
                attp = pair_state[pi]
                for j in range(k):
                    jsl = slice(j * P, (j + 1) * P)
                    nc.tensor.matmul(attp[:, base + j:base + j + 1],
                                     lhsT=m2[:, jsl], rhs=aW[:],
                                     start=True, stop=True)
                for j in range(k):
                    jsl = slice(j * P, (j + 1) * P)
                    nc.tensor.matmul(ps3[:, jsl], lhsT=m2[:, jsl],
                                     rhs=ident_bf[:], start=True, stop=True)
                stt["ps3"] = ps3

            def stage2a2(cis):
                """pair tail: one tanh + att4, then ef4 + S per chunk."""
                pi = cis[0] // 2
                attp = pair_state.pop(pi)
                kk = (cis[-1] % 2) * CHUNK + chunks[cis[-1]][2]
                att_t = wp.tile([128, 8], FP32, tag="att_t")
                nc.scalar.activation(att_t[:, :kk], attp[:, :kk], act_tanh,
                                     bias=ab_c[:], scale=0.5)
                att4 = wp.tile([128, 8], FP32, tag="att4")
                nc.vector.tensor_scalar_add(att4[:, :kk], att_t[:, :kk], 1.0)
                for ci in cis:
                    hf, t0, k = chunks[ci]
                    stt = state[ci]
                    ps3 = stt.pop("ps3")
                    base = (ci % 2) * CHUNK
                    W = k * P
                    ef4 = wp.tile([128, 512], BF, tag="ef4")
                    nc.vector.tensor_tensor(
                        out=ef4[:, :W].rearrange("p (j c) -> p j c", j=k),
                        in0=ps3[:, :W].rearrange("p (j c) -> p j c", j=k),
                        in1=att4[:, base:base + k].to_broadcast([128, k, P]),
                        op=mult)
                    Ss = []
                    for j in range(k):
                        t_idx = t0 + j
                        if j % 2 == 0:
                            S = wp.tile([P, WIN], BF, tag="S")
                            nc.vector.tensor_scalar(
                                out=S[:], in0=jconst[:],
                                scalar1=rel[:, t_idx:t_idx + 1], scalar2=None,
                                op0=iseq)
                        else:
                            S = wp.tile([P, WIN], BF, tag="Sp")
                            nc.gpsimd.tensor_scalar(
                                out=S[:], in0=jconst[:],
                                scalar1=rel[:, t_idx:t_idx + 1], scalar2=None,
                                op0=iseq)
                        Ss.append(S)
                    stt["ef4"] = ef4
                    stt["Ss"] = Ss

            def stage2b(ci):
                hf, t0, k = chunks[ci]
                stt = state.pop(ci)
                ef4 = stt["ef4"]
                for j, S in enumerate(stt["Ss"]):
                    _, w, gfirst, glast = tile_meta[t0 + j]
                    if gfirst:
                        paggT = psg.tile([H, WIN], FP32, space="PSUM",
                                         tag="paggT")
                        pagg_of_group[(hf, w)] = paggT
                    paggT = pagg_of_group[(hf, w)]
                    nc.tensor.matmul(paggT[:], lhsT=ef4[:, j * P:(j + 1) * P],
                                     rhs=S[:], start=gfirst, stop=glast)
                    if glast:
                        del pagg_of_group[(hf, w)]
                        wsl = slice(w * WIN, (w + 1) * WIN)
                        if hf == 0:
                            nc.vector.tensor_copy(aggT32[:, wsl], paggT[:])
                        else:
                            nc.vector.tensor_add(
                                out=aggTbf[:, wsl], in0=aggT32[:, wsl],
                                in1=paggT[:])

            # --- node phase pass (emitted interleaved, 4 windows each) ---
            def node_pass(w0, wg):
                W2 = wg * WIN
                fsl = slice(w0 * WIN, w0 * WIN + W2)
                psn1 = psc.tile([128, 512], FP32, space="PSUM", tag="attp")
                nc.tensor.matmul(psn1[:, :W2], lhsT=nW1t[:], rhs=hT[:, fsl],
                                 start=True, stop=False)
                nc.tensor.matmul(psn1[:, :W2], lhsT=nW1b[:],
                                 rhs=aggTbf[:, fsl], start=False, stop=True)
                y1 = wp.tile([128, 512], BF, tag="m1")
                nc.scalar.activation(y1[:, :W2], psn1[:, :W2], act_silu,
                                     bias=nb1[:])
                psn2 = psc.tile([128, 512], FP32, space="PSUM", tag="attp")
                for j in range(wg):
                    nc.tensor.matmul(psn2[:, j * D:(j + 1) * D],
                                     lhsT=y1[:, j * WIN:(j + 1) * WIN],
                                     rhs=nW2[:], start=True, stop=True)
                o_sb = wp.tile([128, 512], FP32, tag="osb")
                nc.vector.tensor_add(out=o_sb[:, :W2], in0=psn2[:, :W2],
                                     in1=h_own[:, w0 * D:w0 * D + W2])
                for j in range(wg):
                    nc.sync.dma_start(
                        out=out_t[(w0 + j) * WIN:(w0 + j + 1) * WIN, :],
                        in_=o_sb[:, j * D:(j + 1) * D])

            # node group g ready after the chunk finishing window
            # (1, w0+wg-1)'s group; map chunk idx -> node groups to emit
            def chunk_of_tile(t):
                for ci, (hf, t0, k) in enumerate(chunks):
                    if t0 <= t < t0 + k:
                        return ci
                raise AssertionError(t)

            cumB = TA
            last_tile_B = {}
            for w in range(NW):
                cumB += int(T_hw[1, w])
                last_tile_B[w] = cumB - 1
            node_after = {}
            w0 = 0
            while w0 < NW:
                wg = min(4, NW - w0)
                ci_g = chunk_of_tile(last_tile_B[w0 + wg - 1])
                node_after.setdefault(ci_g, []).append((w0, wg))
                w0 += wg

            NC = len(chunks)
            for i in range(NC + 5):
                if i < NC:
                    stage1a(i)
                if 0 <= i - 5:
                    for (nw0, nwg) in node_after.get(i - 5, []):
                        node_pass(nw0, nwg)
                if 0 <= i - 1 < NC:
                    stage2a1(i - 1)
                    ci = i - 1
                    pi = ci // 2
                    last_of_pair = 2 * pi + 1 if 2 * pi + 1 < NC else 2 * pi
                    if ci == last_of_pair:
                        cis = [c for c in (2 * pi, 2 * pi + 1) if c < NC]
                        stage2a2(cis)
                if 0 <= i - 3 < NC:
                    stage2b(i - 3)
                if i < NC:
                    stage1b(i)
                if i == 60:
                    nc.sync.dma_start(out=h_own[:], in_=hown_t[:])
    return nc


def _make_in_maps(prep, inputs):
    eW1 = np.asarray(inputs["eW1"], np.float32)
    aW = np.asarray(inputs["aW"], np.float32)
    nW1 = np.asarray(inputs["nW1"], np.float32)
    nb2 = np.asarray(inputs["nb2"], np.float32).reshape(D)
    jconst = np.broadcast_to(np.arange(WIN, dtype=np.float32)[None, :],
                             (P, WIN))
    common = {
        "hA": prep["hA"], "hB": prep["hB"],
        "eW1top": eW1[:D].astype(BF16), "eW1bot": eW1[D:].astype(BF16),
        "eW2": np.asarray(inputs["eW2"], np.float32).astype(BF16),
        "aW_col": aW.reshape(H, 1).astype(BF16),
        "nW1top": nW1[:D].astype(BF16),
        # att' = tanh+1 = 2*sigmoid  =>  fold the 0.5 in here with 1/NORM
        "nW1bot": (nW1[D:] / (2.0 * NORM)).astype(BF16),
        "nW2": np.asarray(inputs["nW2"], np.float32).astype(BF16),
        "eb1": np.asarray(inputs["eb1"], np.float32).reshape(H, 1),
        "eb2": np.asarray(inputs["eb2"], np.float32).reshape(H, 1),
        "nb1": np.asarray(inputs["nb1"], np.float32).reshape(H, 1),
        "jconst": np.ascontiguousarray(jconst).astype(BF16),
        "ident_bf": np.eye(P, dtype=np.float32).astype(BF16),
        # tanh form: sigmoid(x+ab) = 0.5*tanh(0.5x + 0.5ab) + 0.5
        "ab_c": np.full((P, 1), 0.5 * float(np.asarray(inputs["ab"]).ravel()[0]),
                        dtype=np.float32),
    }
    in_maps = []
    for k in range(NCORES):
        m = dict(common)
        m["col_idx"] = np.ascontiguousarray(prep["col_idx"][k])
        m["rel_row"] = np.ascontiguousarray(prep["rel_row"][k])
        m["ST"] = np.ascontiguousarray(prep["ST"][k])
        # residual carries the nb2 bias (saves the bias matmul on device)
        m["h_own"] = np.ascontiguousarray(
            prep["h_own"][k] + np.tile(nb2, NW)[None, :])
        m["hT"] = np.ascontiguousarray(prep["hT"][k])
        in_maps.append(m)
    return in_maps


_RUN_KW = {}


def kernel(**inputs) -> np.ndarray:
    h = np.asarray(inputs["h"], np.float32)
    prep = _preprocess(h, np.asarray(inputs["edge_index"]))

    nc = bacc.Bacc("TRN2", target_bir_lowering=False, debug=False,
                   num_devices=NCORES)
    _build(nc, prep["NT"], prep["T_hw"],
           act_silu=mybir.ActivationFunctionType.Silu,
           act_tanh=mybir.ActivationFunctionType.Tanh)
    nc.compile()

    in_maps = _make_in_maps(prep, inputs)
    res = bass_utils.run_bass_kernel_spmd(
        nc, in_maps, core_ids=list(range(NCORES)), **_RUN_KW)
    out = np.empty((NPAD, D), dtype=np.float32)
    for k in range(NCORES):
        out[k * SHARD:(k + 1) * SHARD] = np.asarray(res.results[k]["out"])
    kernel._last_results = res
    kernel._last_nc = nc
    return out[:N]


# revision 37
# speedup vs baseline: 203.0991x; 1.0063x over previous
"""GCLConv (GNN message passing) Trainium2 kernel — 8-core edge-parallel, v2.

Strategy:
  - Host: sort edges by (col-half, dest-window); shard by destination node
    range across 8 cores (6272 nodes/core) => no cross-core reduction.
  - Row-side gather ELIMINATED: rows of a tile live in one 128-node window,
    so the layer-1 row term eW1t.T @ h[row] == A_w.T @ ST with
    A_w = h_win @ eW1top (one matmul per window, resident bf16) and ST a
    host-precomputed one-hot [node-in-win x edge] streamed from HBM at
    full DMA efficiency (vs per-edge 256B gather descriptors).
  - 512-wide chunks (4 edge tiles, spanning window groups within a gather
    half) through the edge MLP: 2 silus per chunk on ACT ([128,512]
    PSUM->SBUF); attention tanh batched [128,8] per chunk PAIR.
  - m2 transposed per sub-tile via identity matmul; one wide DVE
    tensor_tensor applies att to the whole chunk (ef4 = m2E * att with a
    per-block stride-0 broadcast AP). One-hot S built by fused DVE/Pool
    tensor_scalar(is_equal) against a per-edge rel scalar (255 sentinel
    for padding edges -> zero column).
  - Aggregation matmul emits aggT [H, win] directly (lhsT=ef4, rhs=S), so
    the node phase needs no PE transposes; node MLP runs 4 windows wide
    and is emitted interleaved as windows complete.
  - Explicit 4-deep software pipeline (stage1a/2a1/2a2/2b/stage1b) with
    per-tag PSUM pools sized to break WAR serialization; small first
    gather batches so compute starts ~12us in.
  - sigmoid via tanh (one ACT table set with Silu/Copy): att' = tanh+1 and
    the 0.5 is folded into nW1bot (= nW1[D:] / NORM / 2).
"""
import sys

sys.path.insert(0, "/opt/trn_rl_repo")

import numpy as np
import ml_dtypes

import concourse.bass as bass
import concourse.bacc as bacc
import concourse.mybir as mybir
import concourse.tile as tile
from concourse import bass_utils

BF16 = ml_dtypes.bfloat16

N = 50000
E = 800000
D = 128
H = 128
P = 128
NCORES = 8
WIN = 128                  # nodes per aggregation window
NW = 49                    # windows per core
SHARD = WIN * NW           # 6272 nodes per core
NPAD = SHARD * NCORES      # 50176
COL_SPLIT = 25088          # col gather table split (both halves < 32768)
COL_HI = NPAD - COL_SPLIT  # 25088
GB = 32                    # max tiles per gather/stream batch
CHUNK = 4                  # tiles per compute chunk (512 edges)
NORM = 100.0

FP32 = mybir.dt.float32
BF = mybir.dt.bfloat16
I16 = mybir.dt.int16


def _idx_layout(idx_flat: np.ndarray) -> np.ndarray:
    """Pack int16 indices into the SWDGE layout [128, n/16]:
    index i -> partition i%16, col i//16, replicated across 8 groups."""
    n = idx_flat.shape[0]
    assert n % 16 == 0
    arr = idx_flat.reshape(n // 16, 16).T.astype(np.int16)  # [16, n/16]
    return np.tile(arr, (8, 1))                             # [128, n/16]


def _preprocess(h: np.ndarray, edge_index: np.ndarray):
    """Build per-core edge tiles + metadata. Returns host data dict."""
    row = np.asarray(edge_index[0], dtype=np.int64)
    col = np.asarray(edge_index[1], dtype=np.int64)

    core_of = row // SHARD
    half_of = (col >= COL_SPLIT).astype(np.int64)
    win_of = (row % SHARD) // WIN

    counts = np.zeros((NCORES, 2, NW), dtype=np.int64)
    np.add.at(counts, (core_of, half_of, win_of), 1)
    tiles_per_group = np.maximum(1, -(-counts // P))        # ceil, min 1
    T_hw = tiles_per_group.max(axis=0)                      # [2, NW] uniform
    NT = int(T_hw.sum())

    col_idx = np.empty((NCORES, 128, NT * 8), dtype=np.int16)
    rel_row = np.empty((NCORES, 128, NT), dtype=np.float32)
    ST = np.empty((NCORES, 128, NT * P), dtype=BF16)
    iota = np.arange(P, dtype=np.float32)[:, None]
    for k in range(NCORES):
        m = core_of == k
        rk, ck, hk, wk = row[m] - k * SHARD, col[m], half_of[m], win_of[m]
        order = np.lexsort((wk, hk))
        rk, ck, hk, wk = rk[order], ck[order], hk[order], wk[order]
        cnt = np.zeros((2, NW), dtype=np.int64)
        np.add.at(cnt, (hk, wk), 1)
        cols_l, rel_l = [], []
        pos = 0
        for hf in range(2):
            for w in range(NW):
                c = int(cnt[hf, w])
                npad_e = int(T_hw[hf, w]) * P - c
                c_g = ck[pos:pos + c] - hf * COL_SPLIT
                rel_g = (rk[pos:pos + c] % WIN).astype(np.float32)
                pos += c
                if npad_e:
                    c_g = np.concatenate([c_g, np.zeros(npad_e, np.int64)])
                    rel_g = np.concatenate(
                        [rel_g, np.full(npad_e, 255.0, np.float32)])
                cols_l.append(c_g)
                rel_l.append(rel_g)
        c_all = np.concatenate(cols_l)
        rel_all = np.concatenate(rel_l)
        assert c_all.shape[0] == NT * P
        col_idx[k] = _idx_layout(c_all.astype(np.int16))
        rel_row[k] = rel_all.reshape(NT, P).T.astype(np.float32)
        ST[k] = (iota == rel_all[None, :]).astype(BF16)

    # gather tables (col side only)
    h_pad = np.zeros((NPAD, D), dtype=np.float32)
    h_pad[:N] = h
    h_bf = h_pad.astype(BF16)
    hA = np.ascontiguousarray(h_bf[:COL_SPLIT])
    hB = np.ascontiguousarray(h_bf[COL_SPLIT:])

    # node-phase buffers per core
    hsh = h_pad.reshape(NCORES, NW, WIN, D)
    h_own = np.ascontiguousarray(
        hsh.transpose(0, 2, 1, 3).reshape(NCORES, WIN, NW * D))  # [128, w*128+d]
    hT = np.ascontiguousarray(
        hsh.transpose(0, 3, 1, 2).reshape(NCORES, D, NW * WIN)).astype(BF16)

    return dict(NT=NT, T_hw=T_hw, col_idx=col_idx, rel_row=rel_row, ST=ST,
                hA=hA, hB=hB, h_own=h_own, hT=hT)


def _build(nc: bass.Bass, NT: int, T_hw: np.ndarray,
           act_silu, act_tanh):
    """Emit the SPMD program. T_hw: [2, NW] tiles per (half, window)."""
    dt = nc.dram_tensor
    hA_t = dt("hA", [COL_SPLIT, D], BF, kind="ExternalInput")
    hB_t = dt("hB", [COL_HI, D], BF, kind="ExternalInput")
    cidx_t = dt("col_idx", [128, NT * 8], I16, kind="ExternalInput")
    rel_t = dt("rel_row", [128, NT], FP32, kind="ExternalInput")
    ST_t = dt("ST", [128, NT * P], BF, kind="ExternalInput")
    hown_t = dt("h_own", [WIN, NW * D], FP32, kind="ExternalInput")
    hT_t = dt("hT", [D, NW * WIN], BF, kind="ExternalInput")
    # weights / consts (replicated)
    eW1t_t = dt("eW1top", [D, H], BF, kind="ExternalInput")
    eW1b_t = dt("eW1bot", [D, H], BF, kind="ExternalInput")
    eW2_t = dt("eW2", [H, H], BF, kind="ExternalInput")
    aW_t = dt("aW_col", [H, 1], BF, kind="ExternalInput")
    nW1t_t = dt("nW1top", [D, H], BF, kind="ExternalInput")
    nW1b_t = dt("nW1bot", [H, H], BF, kind="ExternalInput")      # / (2*NORM)
    nW2_t = dt("nW2", [H, D], BF, kind="ExternalInput")
    eb1_t = dt("eb1", [H, 1], FP32, kind="ExternalInput")
    eb2_t = dt("eb2", [H, 1], FP32, kind="ExternalInput")
    nb1_t = dt("nb1", [H, 1], FP32, kind="ExternalInput")
    jconst_t = dt("jconst", [P, WIN], BF, kind="ExternalInput")
    identb_t = dt("ident_bf", [P, P], BF, kind="ExternalInput")
    ab_t = dt("ab_c", [P, 1], FP32, kind="ExternalInput")  # 0.5*ab

    out_t = dt("out", [SHARD, D], FP32, kind="ExternalOutput")

    # ---- schedule: per-tile metadata, then chunks of <=4 consecutive
    # tiles spanning window groups (within one gather half).
    tile_meta = []  # (hf, w, gfirst, glast) per global tile index
    for hf in range(2):
        for w in range(NW):
            nt = int(T_hw[hf, w])
            for i in range(nt):
                tile_meta.append((hf, w, i == 0, i == nt - 1))
    assert len(tile_meta) == NT
    TA = int(T_hw[0].sum())

    chunks = []   # (hf, t0, k)
    for lo, hi in ((0, TA), (TA, NT)):
        t = lo
        while t < hi:
            k = min(CHUNK, hi - t)
            chunks.append((0 if lo == 0 else 1, t, k))
            t += k

    # pack chunks into gather/stream batches (same half); the first few
    # batches are small so the pipeline starts early, then full GB tiles.
    def batch_cap(bi):
        return (4, 4, 8, 8, 16, 16, 24)[bi] if bi < 7 else GB

    batches = []      # (t0, ntiles, half)
    chunk_batch = {}  # chunk idx -> (batch idx, tile offset within batch)
    first_chunk_of_batch = {}
    for ci, (hf, t0, k) in enumerate(chunks):
        if (not batches or batches[-1][2] != hf
                or batches[-1][1] + k > batch_cap(len(batches) - 1)):
            batches.append((t0, 0, hf))
            first_chunk_of_batch[len(batches) - 1] = ci
        b0, nb, _ = batches[-1]
        chunk_batch[ci] = (len(batches) - 1, nb)
        batches[-1] = (b0, nb + k, hf)

    with tile.TileContext(nc) as tc:
        with (
            tc.tile_pool(name="const", bufs=1) as cp,
            tc.tile_pool(name="gather", bufs=4) as gp,
            tc.tile_pool(name="work", bufs=4) as wp,
            tc.tile_pool(name="psA", bufs=2, space="PSUM") as psa,
            tc.tile_pool(name="psB", bufs=2, space="PSUM") as psb,
            tc.tile_pool(name="psC", bufs=1, space="PSUM") as psc,
            tc.tile_pool(name="psG", bufs=1, space="PSUM") as psg,
                    ):
            # --- resident uploads ---
            def up(shape, dtype, src, tag):
                t = cp.tile(shape, dtype, tag=tag)
                nc.sync.dma_start(out=t[:], in_=src[:])
                return t

            # upload order matters: gather prerequisites first so the first
            # gather/stream batches start ASAP; h_own (node phase only) last.
            CSPLIT = 8 * GB * 8
            cidx = cp.tile([128, NT * 8], I16, tag="cidx")
            nc.sync.dma_start(out=cidx[:, :CSPLIT], in_=cidx_t[:, :CSPLIT])
            hT = up([D, NW * WIN], BF, hT_t, "hT")
            eW1t = up([D, H], BF, eW1t_t, "eW1t")
            eW1b = up([D, H], BF, eW1b_t, "eW1b")
            eW2 = up([H, H], BF, eW2_t, "eW2")
            eb1 = up([H, 1], FP32, eb1_t, "eb1")
            eb2 = up([H, 1], FP32, eb2_t, "eb2")

            # --- gather/stream batches ---
            gtiles = {}

            def emit_batch(bi):
                b0, nb, half = batches[bi]
                src = hA_t if half == 0 else hB_t
                ct = gp.tile([128, GB * P], BF, tag="gcol")
                nc.gpsimd.dma_gather(
                    out_ap=ct[:, :nb * P].rearrange("p (a n) -> p a n", a=1),
                    in_ap=src[:],
                    idxs_ap=cidx[:, b0 * 8:(b0 + nb) * 8],
                    num_idxs=nb * P,
                    num_idxs_reg=nb * P,
                    elem_size=D,
                    transpose=True,
                    single_packet=False,
                )
                st = gp.tile([128, GB * P], BF, tag="gst")
                nc.sync.dma_start(out=st[:, :nb * P],
                                  in_=ST_t[:, b0 * P:(b0 + nb) * P])
                gtiles[bi] = (ct, st)

            emit_batch(0)
            emit_batch(1)
            emit_batch(2)
            nc.sync.dma_start(out=cidx[:, CSPLIT:], in_=cidx_t[:, CSPLIT:])

            aW = up([H, 1], BF, aW_t, "aW")
            rel = up([128, NT], FP32, rel_t, "rel")
            jconst = up([P, WIN], BF, jconst_t, "jconst")
            ident_bf = up([P, P], BF, identb_t, "identb")
            ab_c = up([P, 1], FP32, ab_t, "ab")
            nW1t = up([D, H], BF, nW1t_t, "nW1t")
            nW1b = up([H, H], BF, nW1b_t, "nW1b")
            nW2 = up([H, D], BF, nW2_t, "nW2")
            nb1 = up([H, 1], FP32, nb1_t, "nb1")

            aggT32 = cp.tile([H, NW * WIN], FP32, tag="aggT32")
            aggTbf = cp.tile([H, NW * WIN], BF, tag="aggTbf")
            h_own = cp.tile([WIN, NW * D], FP32, tag="hown")

            mult = mybir.AluOpType.mult
            iseq = mybir.AluOpType.is_equal

            # --- A_w = h_win @ eW1top (bf16 resident), 4 windows per
            # PSUM tile + one wide copy to keep the prologue short ---
            Aw = {}
            for q0 in range(0, NW, 4):
                qn = min(4, NW - q0)
                aps = psa.tile([128, 512], FP32, space="PSUM", tag="ps1")
                for qi in range(qn):
                    wsl = slice((q0 + qi) * WIN, (q0 + qi + 1) * WIN)
                    nc.tensor.matmul(aps[:, qi * H:(qi + 1) * H],
                                     lhsT=hT[:, wsl], rhs=eW1t[:],
                                     start=True, stop=True)
                a_sb = cp.tile([128, 512], BF, tag=f"Aw{q0}")
                nc.vector.tensor_copy(a_sb[:, :qn * H], aps[:, :qn * H])
                for qi in range(qn):
                    Aw[q0 + qi] = a_sb[:, qi * H:(qi + 1) * H]

            def get_Aw(w):
                return Aw[w]
            # --- software pipeline: stage1(i) | stage2a(i-1) | stage2b(i-2)
            state = {}      # chunk idx -> dict of live tiles
            pagg_of_group = {}  # (hf, w) -> paggT tile

            def stage1a(ci):
                hf, t0, k = chunks[ci]
                bi, off = chunk_batch[ci]
                if first_chunk_of_batch.get(bi) == ci:
                    for nb_i in (bi + 1, bi + 2, bi + 3):
                        if nb_i < len(batches) and nb_i not in gtiles:
                            emit_batch(nb_i)
                ct, st = gtiles[bi]
                cs = slice(off * P, (off + k) * P)
                W = k * P
                ps1 = psa.tile([128, 512], FP32, space="PSUM", tag="ps1")
                # one A_w matmul per window-run within the chunk
                j = 0
                while j < k:
                    w = tile_meta[t0 + j][1]
                    j2 = j
                    while j2 < k and tile_meta[t0 + j2][1] == w:
                        j2 += 1
                    nc.tensor.matmul(
                        ps1[:, j * P:j2 * P], lhsT=get_Aw(w),
                        rhs=st[:, off * P + j * P:off * P + j2 * P],
                        start=True, stop=False)
                    j = j2
                nc.tensor.matmul(ps1[:, :W], lhsT=eW1b[:], rhs=ct[:, cs],
                                 start=False, stop=True)
                state[ci] = dict(ps1=ps1, ct=ct, cs=cs)

            def stage1b(ci):
                hf, t0, k = chunks[ci]
                W = k * P
                stt = state[ci]
                ps1 = stt.pop("ps1")
                m1 = wp.tile([128, 512], BF, tag="m1")
                nc.scalar.activation(m1[:, :W], ps1[:, :W], act_silu,
                                     bias=eb1[:])
                ps2 = psa.tile([128, 512], FP32, space="PSUM", tag="ps2")
                nc.tensor.matmul(ps2[:, :W], lhsT=eW2[:], rhs=m1[:, :W],
                                 start=True, stop=True)
                m2 = wp.tile([128, 512], BF, tag="m2")
                nc.scalar.activation(m2[:, :W], ps2[:, :W], act_silu,
                                     bias=eb2[:])
                stt["m2"] = m2

            pair_state = {}

            def stage2a1(ci):
                """ps3 + attp matmuls; attp is shared per chunk pair."""
                hf, t0, k = chunks[ci]
                stt = state[ci]
                m2 = stt["m2"]
                ps3 = psb.tile([128, 512], FP32, space="PSUM", tag="ps3")
                pi, base = ci // 2, (ci % 2) * CHUNK
                if ci % 2 == 0:
                    attp = psc.tile([128, 8], FP32, space="PSUM", tag="attp")
                    pair_state[pi] = attp
                attp = pair_state[pi]
                for j in range(k):
                    jsl = slice(j * P, (j + 1) * P)
                    nc.tensor.matmul(attp[:, base + j:base + j + 1],
                                     lhsT=m2[:, jsl], rhs=aW[:],
                                     start=True, stop=True)
                for j in range(k):
                    jsl = slice(j * P, (j + 1) * P)
                    nc.tensor.matmul(ps3[:, jsl], lhsT=m2[:, jsl],
                                     rhs=ident_bf[:], start=True, stop=True)
                stt["ps3"] = ps3

            def stage2a2(cis):
                """pair tail: one tanh + att4, then ef4 + S per chunk."""
                pi = cis[0] // 2
                attp = pair_state.pop(pi)
                kk = (cis[-1] % 2) * CHUNK + chunks[cis[-1]][2]
                att_t = wp.tile([128, 8], FP32, tag="att_t")
                nc.scalar.activation(att_t[:, :kk], attp[:, :kk], act_tanh,
                                     bias=ab_c[:], scale=0.5)
                att4 = wp.tile([128, 8], FP32, tag="att4")
                nc.vector.tensor_scalar_add(att4[:, :kk], att_t[:, :kk], 1.0)
                for ci in cis:
                    hf, t0, k = chunks[ci]
                    stt = state[ci]
                    ps3 = stt.pop("ps3")
                    base = (ci % 2) * CHUNK
                    W = k * P
                    ef4 = wp.tile([128, 512], BF, tag="ef4")
                    nc.vector.tensor_tensor(
                        out=ef4[:, :W].rearrange("p (j c) -> p j c", j=k),
                        in0=ps3[:, :W].rearrange("p (j c) -> p j c", j=k),
                        in1=att4[:, base:base + k].to_broadcast([128, k, P]),
                        op=mult)
                    Ss = []
                    for j in range(k):
                        t_idx = t0 + j
                        if j % 2 == 0:
                            S = wp.tile([P, WIN], BF, tag="S")
                            nc.vector.tensor_scalar(
                                out=S[:], in0=jconst[:],
                                scalar1=rel[:, t_idx:t_idx + 1], scalar2=None,
                                op0=iseq)
                        else:
                            S = wp.tile([P, WIN], BF, tag="Sp")
                            nc.gpsimd.tensor_scalar(
                                out=S[:], in0=jconst[:],
                                scalar1=rel[:, t_idx:t_idx + 1], scalar2=None,
                                op0=iseq)
                        Ss.append(S)
                    stt["ef4"] = ef4
                    stt["Ss"] = Ss

            def stage2b(ci):
                hf, t0, k = chunks[ci]
                stt = state.pop(ci)
                ef4 = stt["ef4"]
                for j, S in enumerate(stt["Ss"]):
                    _, w, gfirst, glast = tile_meta[t0 + j]
                    if gfirst:
                        paggT = psg.tile([H, WIN], FP32, space="PSUM",
                                         tag="paggT")
                        pagg_of_group[(hf, w)] = paggT
                    paggT = pagg_of_group[(hf, w)]
                    nc.tensor.matmul(paggT[:], lhsT=ef4[:, j * P:(j + 1) * P],
                                     rhs=S[:], start=gfirst, stop=glast)
                    if glast:
                        del pagg_of_group[(hf, w)]
                        wsl = slice(w * WIN, (w + 1) * WIN)
                        if hf == 0:
                            nc.vector.tensor_copy(aggT32[:, wsl], paggT[:])
                        else:
                            nc.vector.tensor_add(
                                out=aggTbf[:, wsl], in0=aggT32[:, wsl],
                                in1=paggT[:])

            # --- node phase pass (emitted interleaved, 4 windows each) ---
            def node_pass(w0, wg):
                W2 = wg * WIN
                fsl = slice(w0 * WIN, w0 * WIN + W2)
                psn1 = psc.tile([128, 512], FP32, space="PSUM", tag="attp")
                nc.tensor.matmul(psn1[:, :W2], lhsT=nW1t[:], rhs=hT[:, fsl],
                                 start=True, stop=False)
                nc.tensor.matmul(psn1[:, :W2], lhsT=nW1b[:],
                                 rhs=aggTbf[:, fsl], start=False, stop=True)
                y1 = wp.tile([128, 512], BF, tag="m1")
                nc.scalar.activation(y1[:, :W2], psn1[:, :W2], act_silu,
                                     bias=nb1[:])
                psn2 = psc.tile([128, 512], FP32, space="PSUM", tag="attp")
                for j in range(wg):
                    nc.tensor.matmul(psn2[:, j * D:(j + 1) * D],
                                     lhsT=y1[:, j * WIN:(j + 1) * WIN],
                                     rhs=nW2[:], start=True, stop=True)
                o_sb = wp.tile([128, 512], FP32, tag="osb")
                nc.vector.tensor_add(out=o_sb[:, :W2], in0=psn2[:, :W2],
                                     in1=h_own[:, w0 * D:w0 * D + W2])
                for j in range(wg):
                    nc.sync.dma_start(
                        out=out_t[(w0 + j) * WIN:(w0 + j + 1) * WIN, :],
                        in_=o_sb[:, j * D:(j + 1) * D])

            # node group g ready after the chunk finishing window
            # (1, w0+wg-1)'s group; map chunk idx -> node groups to emit
            def chunk_of_tile(t):
                for ci, (hf, t0, k) in enumerate(chunks):
                    if t0 <= t < t0 + k:
                        return ci
                raise AssertionError(t)

            cumB = TA
            last_tile_B = {}
            for w in range(NW):
                cumB += int(T_hw[1, w])
                last_tile_B[w] = cumB - 1
            node_after = {}
            w0 = 0
            while w0 < NW:
                wg = min(4, NW - w0)
                ci_g = chunk_of_tile(last_tile_B[w0 + wg - 1])
                node_after.setdefault(ci_g, []).append((w0, wg))
                w0 += wg

            NC = len(chunks)
            for i in range(NC + 7):
                if i < NC:
                    stage1a(i)
                if 0 <= i - 6:
                    for (nw0, nwg) in node_after.get(i - 6, []):
                        node_pass(nw0, nwg)
                if 0 <= i - 1 < NC:
                    stage2a1(i - 1)
                    ci = i - 1
                    pi = ci // 2
                    last_of_pair = 2 * pi + 1 if 2 * pi + 1 < NC else 2 * pi
                    if ci == last_of_pair:
                        cis = [c for c in (2 * pi, 2 * pi + 1) if c < NC]
                        stage2a2(cis)
                if 0 <= i - 3 < NC:
                    stage2b(i - 3)
                if i < NC:
                    stage1b(i)
                if i == 60:
                    nc.sync.dma_start(out=h_own[:], in_=hown_t[:])
    return nc


def _make_in_maps(prep, inputs):
    eW1 = np.asarray(inputs["eW1"], np.float32)
    aW = np.asarray(inputs["aW"], np.float32)
    nW1 = np.asarray(inputs["nW1"], np.float32)
    nb2 = np.asarray(inputs["nb2"], np.float32).reshape(D)
    jconst = np.broadcast_to(np.arange(WIN, dtype=np.float32)[None, :],
                             (P, WIN))
    common = {
        "hA": prep["hA"], "hB": prep["hB"],
        "eW1top": eW1[:D].astype(BF16), "eW1bot": eW1[D:].astype(BF16),
        "eW2": np.asarray(inputs["eW2"], np.float32).astype(BF16),
        "aW_col": aW.reshape(H, 1).astype(BF16),
        "nW1top": nW1[:D].astype(BF16),
        # att' = tanh+1 = 2*sigmoid  =>  fold the 0.5 in here with 1/NORM
        "nW1bot": (nW1[D:] / (2.0 * NORM)).astype(BF16),
        "nW2": np.asarray(inputs["nW2"], np.float32).astype(BF16),
        "eb1": np.asarray(inputs["eb1"], np.float32).reshape(H, 1),
        "eb2": np.asarray(inputs["eb2"], np.float32).reshape(H, 1),
        "nb1": np.asarray(inputs["nb1"], np.float32).reshape(H, 1),
        "jconst": np.ascontiguousarray(jconst).astype(BF16),
        "ident_bf": np.eye(P, dtype=np.float32).astype(BF16),
        # tanh form: sigmoid(x+ab) = 0.5*tanh(0.5x + 0.5ab) + 0.5
        "ab_c": np.full((P, 1), 0.5 * float(np.asarray(inputs["ab"]).ravel()[0]),
                        dtype=np.float32),
    }
    in_maps = []
    for k in range(NCORES):
        m = dict(common)
        m["col_idx"] = np.ascontiguousarray(prep["col_idx"][k])
        m["rel_row"] = np.ascontiguousarray(prep["rel_row"][k])
        m["ST"] = np.ascontiguousarray(prep["ST"][k])
        # residual carries the nb2 bias (saves the bias matmul on device)
        m["h_own"] = np.ascontiguousarray(
            prep["h_own"][k] + np.tile(nb2, NW)[None, :])
        m["hT"] = np.ascontiguousarray(prep["hT"][k])
        in_maps.append(m)
    return in_maps


_RUN_KW = {}


def kernel(**inputs) -> np.ndarray:
    h = np.asarray(inputs["h"], np.float32)
    prep = _preprocess(h, np.asarray(inputs["edge_index"]))

    nc = bacc.Bacc("TRN2", target_bir_lowering=False, debug=False,
                   num_devices=NCORES)
    _build(nc, prep["NT"], prep["T_hw"],
           act_silu=mybir.ActivationFunctionType.Silu,
           act_tanh=mybir.ActivationFunctionType.Tanh)
    nc.compile()

    in_maps = _make_in_maps(prep, inputs)
    res = bass_utils.run_bass_kernel_spmd(
        nc, in_maps, core_ids=list(range(NCORES)), **_RUN_KW)
    out = np.empty((NPAD, D), dtype=np.float32)
    for k in range(NCORES):
        out[k * SHARD:(k + 1) * SHARD] = np.asarray(res.results[k]["out"])
    kernel._last_results = res
    kernel._last_nc = nc
    return out[:N]
